# revision 21
# baseline (speedup 1.0000x reference)
"""ArcDecoder distributed Bass kernel for 8 TRN2 NeuronCores.

Problem: for each arc e with endpoints (s, d):
    h   = concat(z[s], z[d])                # [256]
    h1  = relu(W1 @ h + b1)                 # [128]
    out = W2 @ h1 + b2                      # scalar

Strategy (dense, host-expanded): the host pre-gathers the endpoint
embeddings into two dense streams per core, zs = z[src].T and
zd = z[dst].T, each [128, E_core] in fp8-e4m3 (validated: norm rel err
~1.5e-2 < 2e-2 gate).  With W1 split as [W1a | W1b], folded with |W2|,
and j-columns reordered so all sgn(W2)=+1 columns come first (k of them;
k is a compile-time constant since the graph is built per call):
    psum[slot, j] = zs_tile @ wa + zd_tile @ wb        (PE, psum add)
    rs            = relu(psum)                         (ACT, psum->bf16)
    outP[slot]    = sum_{j<k} rs,  outN[slot] = sum_{j>=k} rs
                    (column pairs pre-folded on GPSIMD, final reduce on
                     DVE; every 4th chunk reduces unfolded on DVE to
                     balance the two engines)
    out[slot]     = outP - outN + b2                   (host)
Fully dense streaming: big sequential DMAs, no gathers, no tables.
b1 is folded into zs via a host-side shift c solving wa.T c = |W2|*b1
(dormant here since b1 = 0); b2 and the P-N subtract run on the host.

Sharding: arcs split evenly across the 8 cores; weights replicated.
No collectives.
"""

import numpy as np

# ---------------- problem constants (hardcoded, per the task spec) ----------
N_NODES = 100000
HIDDEN = 128
N_ARCS = 1000000
N_CORES = 8

P = 128  # SBUF partitions

E_PER_CORE = N_ARCS // N_CORES  # 125000
NT = 992                        # slot tiles per core (992*128 = 126976 slots)
E_PAD = NT * P

PCHUNK = 16   # tiles per psum chunk (16*128 f32 = 8KB/partition = 4 banks)
DCHUNK = 32   # tiles per input-DMA chunk (512 KB per fp8 stream)
N_DC = NT // DCHUNK  # 31

Z_FP8 = True  # z streams in fp8-e4m3 (else bf16)


def _build_graph(k_pos, z_fp8=Z_FP8):
    """Build the SPMD single-core graph (all 8 cores run this same graph).

    k_pos: number of leading j-columns with sgn(W2) = +1 (rest negative).
    """
    import concourse.bass as bass
    from concourse import bacc, mybir, tile

    BF16 = mybir.dt.bfloat16
    F32 = mybir.dt.float32
    ZDT = mybir.dt.float8e4 if z_fp8 else BF16

    nc = bacc.Bacc(None, target_bir_lowering=False)
    with tile.TileContext(nc) as tc:
        with tc.tile_pool(name="dram", bufs=1, space="DRAM") as dram:
            zs_d = dram.tile([P, E_PAD], ZDT, kind="ExternalInput",
                             name="zs", uniquify=False)
            zd_d = dram.tile([P, E_PAD], ZDT, kind="ExternalInput",
                             name="zd", uniquify=False)
            wa_d = dram.tile([P, P], BF16, kind="ExternalInput",
                             name="wa", uniquify=False)
            wb_d = dram.tile([P, P], BF16, kind="ExternalInput",
                             name="wb", uniquify=False)
            F16 = mybir.dt.float16
            outm = dram.tile([P, 2 * NT], F16, kind="ExternalOutput",
                             name="outm", uniquify=False)

            with tc.tile_pool(name="consts", bufs=1) as cpool:
                wa_s = cpool.tile([P, P], BF16, name="wa_s")
                nc.sync.dma_start(out=wa_s[:], in_=wa_d[:])
                wb_s = cpool.tile([P, P], BF16, name="wb_s")
                nc.sync.dma_start(out=wb_s[:], in_=wb_d[:])
                GCH = 8  # chunks per result-flush group (8*PCHUNK=128 cols)

                with tc.tile_pool(name="zin", bufs=3) as zpool, \
                     tc.tile_pool(name="ps", bufs=2, space="PSUM") as pspool, \
                     tc.tile_pool(name="res", bufs=2) as respool, \
                     tc.tile_pool(name="rs", bufs=4) as rspool:
                    gP = gN = None
                    for c2 in range(N_DC):
                        zs_t = zpool.tile([P, DCHUNK * P], ZDT, tag="zs")
                        nc.sync.dma_start(
                            out=zs_t[:],
                            in_=zs_d[:, c2 * DCHUNK * P:(c2 + 1) * DCHUNK * P])
                        zd_t = zpool.tile([P, DCHUNK * P], ZDT, tag="zd")
                        nc.sync.dma_start(
                            out=zd_t[:],
                            in_=zd_d[:, c2 * DCHUNK * P:(c2 + 1) * DCHUNK * P])
                        for h in range(DCHUNK // PCHUNK):
                            c = c2 * (DCHUNK // PCHUNK) + h
                            ps = pspool.tile([P, PCHUNK * P], F32, tag="ps")
                            for t in range(PCHUNK):
                                f0 = (h * PCHUNK + t) * P
                                nc.tensor.matmul(ps[:, t * P:(t + 1) * P],
                                                 lhsT=zs_t[:, f0:f0 + P],
                                                 rhs=wa_s[:],
                                                 start=True, stop=False)
                                nc.tensor.matmul(ps[:, t * P:(t + 1) * P],
                                                 lhsT=zd_t[:, f0:f0 + P],
                                                 rhs=wb_s[:],
                                                 start=False, stop=True)
                            rs = rspool.tile([P, PCHUNK, P], BF16, tag="rs")
                            nc.scalar.activation(
                                out=rs[:].rearrange("p t j -> p (t j)"),
                                in_=ps[:],
                                func=mybir.ActivationFunctionType.Relu)
                            # halve the DVE reduce: fold column pairs within
                            # each sign zone on the (idle) gpsimd engine.
                            # zone [0:k): fold [hp:k) onto [0:k-hp); odd
                            # middle column hp-1 stays put inside [0:hp).
                            k, m = k_pos, P - k_pos
                            fold = c % 10 < 7  # 30% of chunks: DVE direct
                            hp = (k + 1) // 2 if fold else k
                            hm = (m + 1) // 2 if fold else m
                            if c % GCH == 0:
                                n_ch = NT // PCHUNK
                                gW = min(GCH, n_ch - c) * PCHUNK
                                gP = respool.tile([P, GCH * PCHUNK], F16,
                                                  tag="gP")
                                gN = respool.tile([P, GCH * PCHUNK], F16,
                                                  tag="gN")
                                if k_pos == 0:
                                    nc.vector.memset(gP[:], 0.0)
                                if k_pos == P:
                                    nc.vector.memset(gN[:], 0.0)
                            o0 = (c % GCH) * PCHUNK
                            if fold and k > 1:
                                nc.gpsimd.tensor_tensor(
                                    out=rs[:, :, 0:k - hp],
                                    in0=rs[:, :, 0:k - hp],
                                    in1=rs[:, :, hp:k],
                                    op=mybir.AluOpType.add)
                            if fold and m > 1:
                                nc.gpsimd.tensor_tensor(
                                    out=rs[:, :, k:k + m - hm],
                                    in0=rs[:, :, k:k + m - hm],
                                    in1=rs[:, :, k + hm:P],
                                    op=mybir.AluOpType.add)
                            with nc.allow_low_precision("f16 partials"):
                                if k > 0:
                                    nc.vector.tensor_reduce(
                                        out=gP[:, o0:o0 + PCHUNK],
                                        in_=rs[:, :, 0:hp],
                                        axis=mybir.AxisListType.X,
                                        op=mybir.AluOpType.add)
                                if m > 0:
                                    nc.vector.tensor_reduce(
                                        out=gN[:, o0:o0 + PCHUNK],
                                        in_=rs[:, :, k:k + hm],
                                        axis=mybir.AxisListType.X,
                                        op=mybir.AluOpType.add)
                            if c % GCH == GCH - 1 or c == NT // PCHUNK - 1:
                                gb = (c // GCH) * GCH * PCHUNK
                                nc.sync.dma_start(
                                    out=outm[:, gb:gb + gW],
                                    in_=gP[:, 0:gW])
                                nc.sync.dma_start(
                                    out=outm[:, NT + gb:NT + gb + gW],
                                    in_=gN[:, 0:gW])
    nc.compile()
    return nc


def _host_prep(z, pot_arcs, W1, b1, W2, b2, n_cores=N_CORES, z_fp8=Z_FP8):
    """Stage inputs: fold weights, reorder j by sign, expand embeddings."""
    import ml_dtypes

    bf16 = ml_dtypes.bfloat16
    zdt = ml_dtypes.float8_e4m3 if z_fp8 else bf16
    H = HIDDEN
    z = np.asarray(z, np.float32)
    W1 = np.asarray(W1, np.float32)
    b1 = np.asarray(b1, np.float32).reshape(-1)
    W2 = np.asarray(W2, np.float32).reshape(-1)
    b2 = np.asarray(b2, np.float32).reshape(-1)
    arcs = np.asarray(pot_arcs)

    absw2 = np.abs(W2)
    sgn = np.sign(W2)
    # reorder j: positive-sgn columns first (zero-sgn columns are inert
    # since |W2|=0 there; count them as "positive")
    order = np.argsort(sgn < 0, kind="stable")
    k_pos = int((sgn >= 0).sum())
    wa = np.ascontiguousarray((W1[:, :H] * absw2[:, None]).T[:, order])
    wb = np.ascontiguousarray((W1[:, H:] * absw2[:, None]).T[:, order])

    zT = np.ascontiguousarray(z.T)  # [128, N] f32
    zsrc_shift = None
    if np.any(b1):
        # fold b1: psum += |W2|*b1 via zs += c with wa.T @ c = (|W2|*b1)[order]
        beta = (absw2 * b1)[order].astype(np.float64)
        c = np.linalg.solve(np.asarray(wa, np.float64).T, beta)
        zsrc_shift = c.astype(np.float32)

    wa16 = wa.astype(bf16)
    wb16 = wb.astype(bf16)

    src = np.asarray(arcs[:, 0], np.int64)
    dst = np.asarray(arcs[:, 1], np.int64)
    in_maps = []
    for ci in range(n_cores):
        lo, hi = ci * E_PER_CORE, (ci + 1) * E_PER_CORE
        s_idx = np.zeros(E_PAD, np.int64)
        d_idx = np.zeros(E_PAD, np.int64)
        s_idx[:E_PER_CORE] = src[lo:hi]
        d_idx[:E_PER_CORE] = dst[lo:hi]
        zs = zT[:, s_idx]  # [128, E_PAD] f32
        zd = zT[:, d_idx]
        if zsrc_shift is not None:
            zs = zs + zsrc_shift[:, None]
        in_maps.append(dict(
            zs=np.ascontiguousarray(zs).astype(zdt),
            zd=np.ascontiguousarray(zd).astype(zdt),
            wa=wa16, wb=wb16))
    return in_maps, float(b2[0]), k_pos


def _assemble(results, b2_val):
    """results[c]["outm"] is [128, 2*NT] f32: [resP | resN] columns."""
    out = np.empty(N_ARCS, np.float32)
    for c in range(N_CORES):
        dev = np.asarray(results[c]["outm"], np.float32)
        val = dev[:, :NT] - dev[:, NT:]
        out[c * E_PER_CORE:(c + 1) * E_PER_CORE] = \
            val.T.reshape(-1)[:E_PER_CORE]
    return out + b2_val


_GRAPH_CACHE = {}


def _get_graph(k_pos):
    key = (k_pos, Z_FP8)
    if key not in _GRAPH_CACHE:
        _GRAPH_CACHE[key] = _build_graph(k_pos)
    return _GRAPH_CACHE[key]


def kernel(z, pot_arcs, W1, b1, W2, b2):
    from concourse.bass_utils import run_bass_kernel_spmd

    in_maps, b2_val, k_pos = _host_prep(z, pot_arcs, W1, b1, W2, b2)
    nc = _get_graph(k_pos)
    res = run_bass_kernel_spmd(nc, in_maps, core_ids=list(range(N_CORES)))
    return _assemble(res.results, b2_val)
